# revision 11
# baseline (speedup 1.0000x reference)
"""Trainium2 Bass kernel for nn_Net_58033598104011 (two-level NNConv GNN).

Instruction-count-minimized redesign:
- conv MLPs h=relu(ea@W+b) built with block-batched broadcast tensor ops
  (no per-subtile matmuls/activations).
- Edge aggregation via one matmul per 128-edge subtile in the
  (dst-node-partition, feature) orientation: U += sel^T @ u.
- Degree normalization folded into the gathered-x operand per edge;
  root/bias terms accumulated directly into the same PSUM tile.
- conv2: per-cluster rows [xp | t | posp] (t = posp @ w2a[:3]) gathered once
  per edge; cartesian MLP input reconstructed as rec*(t_s - t_d) + c_row so
  the cart pass and the conv pass share one gather; dst-side rows come from a
  local selT matmul instead of a second gather.
- Pool max via single strided reduce_max over gathered layers.
- All index/schedule tensors loaded once and kept SBUF-resident.
Host side: index crunching, sorting, schedules, and input-array permutations
only; all FP math on x/edge_attr/pos/weights runs on device.
"""
import sys
sys.path.insert(0, '/opt/trn_rl_repo')
import numpy as np

import concourse.bass as bass
import concourse.mybir as mybir
import concourse.tile as tile
from concourse.bass import compact_to_ranges
from concourse.masks import make_identity
from concourse.vector_clock import ScopedClock

F32 = mybir.dt.float32
I32 = mybir.dt.int32
AX = mybir.AxisListType.X
AXY = mybir.AxisListType.XY
OP = mybir.AluOpType
ACT = mybir.ActivationFunctionType

# ---------------------------------------------------------------------------
# walrus workaround: this toolchain rejects instructions with >1 sync waits on
# the tail drain; split waits onto single-wait nops and chunk sem resets.
# ---------------------------------------------------------------------------

def _patched_drain_and_barrier(self, tick_clock, wait_clock):
    import bass_rust
    nc = self.nc
    drain_inst = nc.sync.drain()
    wait_clock.add_sem_waits(
        drain_inst.ins, ScopedClock({None: tick_clock.global_clock})
    )
    si = drain_inst.ins.sync_info
    waits = list(si.on_wait or []) if si is not None else []
    if len(waits) > 1:
        si.on_wait = waits[:1]
        for w in waits[1:]:
            assert w.wait_mode == 'sem-ge-imm', w
            nop = nc.sync.nop()
            nop._wait_ge(bass_rust.SemaphoreHandle(w.ant_name, w.id), w.wait_value)
    nc.all_engine_barrier()
    assert self.sems is not None
    popped = nc._tile_sem_poison_stack.pop()
    assert popped is self._sem_poison
    nc.clear_and_free_semaphores(list(self.sems.allocated().values()))
    nc.all_engine_barrier()


def _patched_clear_and_free(self, sems):
    if not sems:
        return
    sem_nums = [s.num if hasattr(s, 'num') else s for s in sems]
    for sem_range in compact_to_ranges(sem_nums):
        lo, hi = sem_range.start, sem_range.stop
        for s in range(lo, hi, 64):
            sub = range(s, min(s + 64, hi))
            assert self._state.free_isdisjoint(sub)
            self.gpsimd.dma_reset(sub)
            self.gpsimd.sem_clear(sub)
    self._state.prepend_free_semaphores(sem_nums)
    for poison_set in self._tile_sem_poison_stack:
        poison_set.update(sem_nums)


def install_tilefix():
    tile.TileContext._drain_and_barrier = _patched_drain_and_barrier
    bass.Bass.clear_and_free_semaphores = _patched_clear_and_free


def split_excess_waits(nc, limit=2):
    """walrus in this container accepts only `limit` sync waits per
    instruction; hoist the rest onto same-engine nops placed just before."""
    import bass_rust
    for fn in nc.m.functions:
        for bb in fn.blocks:
            insts = list(bb.instructions)
            out = []
            changed = False
            for inst in insts:
                si = inst.sync_info
                waits = list(si.on_wait or []) if si is not None else []
                if len(waits) > limit:
                    eq = [w for w in waits if w.wait_mode != 'sem-ge-imm']
                    ge = [w for w in waits if w.wait_mode == 'sem-ge-imm']
                    assert len(eq) <= limit, (inst.name, eq)
                    ordered = eq + ge
                    keep, hoist = ordered[:limit], ordered[limit:]
                    eng = nc.engines[inst.engine]
                    for w in hoist:
                        nop = eng.nop()
                        cur = list(nc.cur_bb.bb.instructions)
                        assert cur[-1].name == nop.ins.name
                        nc.cur_bb.bb.instructions = cur[:-1]
                        nop._wait_ge(
                            bass_rust.SemaphoreHandle(w.ant_name, w.id),
                            w.wait_value)
                        out.append(nop.ins)
                    si.on_wait = keep
                    changed = True
                out.append(inst)
            if changed:
                bb.instructions = out


# ---------------------------------------------------------------------------
# host-side prep: all index crunching, sharding, schedules
# ---------------------------------------------------------------------------

def _ceil(a, b):
    return -(-a // b)


def _pad128(n):
    return _ceil(n, 128) * 128


def prep(inputs, R=8):
    x = np.asarray(inputs["x"], np.float32)
    ea = np.asarray(inputs["edge_attr"], np.float32)
    pos = np.asarray(inputs["pos"], np.float32)
    ei = np.asarray(inputs["edge_index"], np.int64).astype(np.int32)
    batch = np.asarray(inputs["batch"], np.int64).astype(np.int32)
    cl1 = np.asarray(inputs["cluster1"], np.int64).astype(np.int32)
    ei2 = np.asarray(inputs["edge_index2"], np.int64).astype(np.int32)
    cl2 = np.asarray(inputs["cluster2"], np.int64).astype(np.int32)

    N, FV = x.shape
    E, FE = ea.shape
    C1 = int(cl1.max()) + 1 if cl1.size else 1
    C1 = max(C1, int(ei2.max()) + 1 if ei2.size else 1, cl2.shape[0])
    C2 = int(cl2.max()) + 1
    E2 = ei2.shape[1]
    B = int(batch.max()) + 1
    h1 = inputs["w1a"].shape[1]          # 25
    co1 = inputs["root1"].shape[1]       # 32
    ci2, co2 = inputs["root2"].shape     # 32, 64
    NCLS = inputs["fc2_w"].shape[1]      # 10
    FCH = inputs["fc1_w"].shape[1]       # 128

    NS = _pad128(_ceil(N, R))
    CS = _pad128(_ceil(C1, R))
    C2S = _pad128(_ceil(C2, R))
    NP, C1P, C2P = R * NS, R * CS, R * C2S
    NB1, NBP, NB2 = NS // 128, CS // 128, C2S // 128

    RW = 64                      # conv2 row: [xp(32)|t(25)|posp(4)|pad(3)]
    L1 = 32 * (NS + 1)           # x1 rows + one -1 pad row per rank
    L2 = RW * (CS + 1)           # [xp|t|posp] rows + zero pad row
    L25 = 64 * (CS + 1)          # x2 rows + pad row
    L3 = 64 * B
    sent1 = NS                   # rank-0 pad row idx in e1o row view
    sent2 = CS                   # rank-0 pad row idx in e25o row view

    p = dict(R=R, N=N, E=E, C1=C1, C2=C2, E2=E2, B=B, FV=FV, FE=FE,
             h1=h1, co1=co1, ci2=ci2, co2=co2, NCLS=NCLS, FCH=FCH,
             NS=NS, CS=CS, C2S=C2S, NB1=NB1, NBP=NBP, NB2=NB2,
             L1=L1, L2=L2, L25=L25, L3=L3, RW=RW)

    # ---- weights ----
    w1a_aug = np.vstack([np.asarray(inputs["w1a"], np.float32),
                         np.asarray(inputs["b1a"], np.float32)[None]])  # (4,25)
    w2a_aug = np.vstack([np.asarray(inputs["w2a"], np.float32),
                         np.asarray(inputs["b2a"], np.float32)[None]])  # (4,25)

    def make_wbig(wb, bb, ci, co):
        wb = np.asarray(wb, np.float32)    # (h1, ci*co)
        bb = np.asarray(bb, np.float32)    # (ci*co,)
        W = np.empty((ci * (h1 + 1), co), np.float32)
        for i in range(ci):
            W[i * (h1 + 1): i * (h1 + 1) + h1, :] = wb[:, i * co:(i + 1) * co]
            W[i * (h1 + 1) + h1, :] = bb[i * co:(i + 1) * co]
        return W

    wbig1 = make_wbig(inputs["w1b"], inputs["b1b"], FV, co1)     # (156,32)
    wbig2 = make_wbig(inputs["w2b"], inputs["b2b"], ci2, co2)    # (832,64)
    root1_aug = np.vstack([np.asarray(inputs["root1"], np.float32),
                           np.asarray(inputs["bias1"], np.float32)[None]])  # (7,32)
    root2_aug = np.vstack([np.asarray(inputs["root2"], np.float32),
                           np.asarray(inputs["bias2"], np.float32)[None]])  # (33,64)

    shared = dict(
        w1a_aug=w1a_aug, w2a_aug=w2a_aug, wbig1=wbig1, wbig2=wbig2,
        root1_aug=root1_aug, root2_aug=root2_aug,
        fc1_w=np.asarray(inputs["fc1_w"], np.float32),
        fc1_b=np.asarray(inputs["fc1_b"], np.float32).reshape(FCH, 1),
        fc2_w=np.asarray(inputs["fc2_w"], np.float32),
        fc2_b=np.asarray(inputs["fc2_b"], np.float32).reshape(NCLS, 1),
    )
    x_em = np.zeros((NP, 8), np.float32); x_em[:N, :FV] = x
    pos_em = np.zeros((NP, 4), np.float32); pos_em[:N, :3] = pos
    shared["x_em"] = x_em
    shared["halfones"] = np.array([[0.5], [0.5], [0.5], [1.0]], np.float32)

    xaugT_full = np.zeros((FV + 1, NP), np.float32)
    xaugT_full[:FV, :N] = x.T
    xaugT_full[FV, :] = 1.0

    # ---- conv1 schedule: edges sorted by dst, sharded by dst range ----
    src, dst = ei[0], ei[1]
    order = np.argsort(dst, kind='stable')
    s_src, s_dst, s_ea = src[order], dst[order], ea[order]
    deg = np.bincount(dst, minlength=NP).astype(np.float32)
    dinv_full = (1.0 / np.maximum(deg, 1.0)).astype(np.float32)
    dinv_of_edge = dinv_full[s_dst]

    blk_edges = [[None] * NB1 for _ in range(R)]
    for r in range(R):
        for b in range(NB1):
            lo = r * NS + b * 128
            hi = lo + 128
            blk_edges[r][b] = (np.searchsorted(s_dst, lo),
                               np.searchsorted(s_dst, hi))
    S1 = [max(1, max(_ceil(blk_edges[r][b][1] - blk_edges[r][b][0], 128)
                     for r in range(R))) for b in range(NB1)]
    S1tot = sum(S1)
    eabE = np.zeros((R, 128, 3 * S1tot), np.float32)
    src1i = np.zeros((R, 128, S1tot), np.int32)
    dst1loc = np.full((R, 128, S1tot), -1, np.int32)
    dinv1E = np.zeros((R, 128, S1tot), np.float32)
    t0 = 0
    for b in range(NB1):
        for r in range(R):
            i0, i1 = blk_edges[r][b]
            ne = i1 - i0
            nn = 128 * S1[b]
            col = np.zeros(nn, np.int32)
            dl = np.full(nn, -1, np.int32)
            dvv = np.zeros(nn, np.float32)
            eaa = np.zeros((nn, 3), np.float32)
            col[:ne] = s_src[i0:i1]
            dl[:ne] = s_dst[i0:i1] - (r * NS + b * 128)
            dvv[:ne] = dinv_of_edge[i0:i1]
            eaa[:ne] = s_ea[i0:i1]
            # subtile-major strips: subtile s holds edges [s*128:(s+1)*128]
            eabE[r, :, 3 * t0:3 * (t0 + S1[b])] = (
                eaa.reshape(S1[b], 128, 3).transpose(1, 0, 2).reshape(128, -1))
            src1i[r, :, t0:t0 + S1[b]] = col.reshape(S1[b], 128).T
            dst1loc[r, :, t0:t0 + S1[b]] = dl.reshape(S1[b], 128).T
            dinv1E[r, :, t0:t0 + S1[b]] = dvv.reshape(S1[b], 128).T
        t0 += S1[b]
    p["S1"] = S1

    percore = dict(
        eabE=eabE, src1i=src1i, dst1loc=dst1loc, dinv1E=dinv1E,
        xaugT=np.stack([xaugT_full[:, r * NS:(r + 1) * NS] for r in range(R)]),
    )

    # ---- posp schedule: nodes sorted by cluster1, sharded by cluster range ----
    corder = np.argsort(cl1, kind='stable')
    c_nodes, c_cl = corder.astype(np.int32), cl1[corder]
    csize = np.bincount(cl1, minlength=C1P).astype(np.float32)
    cinv_of_node = (1.0 / np.maximum(csize, 1.0))[c_cl]

    pblk = [[None] * NBP for _ in range(R)]
    for r in range(R):
        for b in range(NBP):
            lo, hi = r * CS + b * 128, r * CS + (b + 1) * 128
            pblk[r][b] = (np.searchsorted(c_cl, lo), np.searchsorted(c_cl, hi))
    SP = [max(1, max(_ceil(pblk[r][b][1] - pblk[r][b][0], 128)
                     for r in range(R))) for b in range(NBP)]
    SPtot = sum(SP)
    pos_cl = np.zeros((R, 128, 4 * SPtot), np.float32)   # host-permuted pos
    clloc = np.full((R, 128, SPtot), -1, np.int32)
    wcnt = np.zeros((R, 128, SPtot), np.float32)
    t0 = 0
    for b in range(NBP):
        for r in range(R):
            i0, i1 = pblk[r][b]
            nn_ = i1 - i0
            nn = 128 * SP[b]
            pp = np.zeros((nn, 4), np.float32)
            cc = np.full(nn, -1, np.int32)
            wc = np.zeros(nn, np.float32)
            pp[:nn_] = pos_em[c_nodes[i0:i1]]
            cc[:nn_] = c_cl[i0:i1] - (r * CS + b * 128)
            wc[:nn_] = cinv_of_node[i0:i1]
            pos_cl[r, :, 4 * t0:4 * (t0 + SP[b])] = (
                pp.reshape(SP[b], 128, 4).transpose(1, 0, 2).reshape(128, -1))
            clloc[r, :, t0:t0 + SP[b]] = cc.reshape(SP[b], 128).T
            wcnt[r, :, t0:t0 + SP[b]] = wc.reshape(SP[b], 128).T
        t0 += SP[b]
    p["SP"] = SP
    percore.update(pos_cl=pos_cl, clloc=clloc, wcnt=wcnt)

    # ---- pool1 layered gather schedule ----
    def x1row(n):
        r = n // NS
        return r * (NS + 1) + (n - r * NS)

    K1 = []
    lay1 = [[] for _ in range(R)]
    for b in range(NBP):
        kb = 1
        tabs = []
        for r in range(R):
            i0, i1 = pblk[r][b]
            nodes, cls = c_nodes[i0:i1], c_cl[i0:i1] - (r * CS + b * 128)
            tab = {}
            for n_, c_ in zip(nodes, cls):
                tab.setdefault(int(c_), []).append(int(n_))
            tabs.append(tab)
            if tab:
                kb = max(kb, max(len(v) for v in tab.values()))
        K1.append(kb)
        for r in range(R):
            lt = np.full((kb, 128), sent1, np.int64)
            for c_, ns_ in tabs[r].items():
                for j, n_ in enumerate(ns_):
                    lt[j, c_] = x1row(n_)
            lay1[r].append(lt)
    K1tot = sum(K1)
    xp1i = np.stack([np.concatenate(lay1[r], 0).T.astype(np.int32)
                     for r in range(R)])
    p["K1"] = K1
    xmskB = np.zeros((R, 128, NBP), np.float32)
    for r in range(R):
        for b in range(NBP):
            xmskB[r, :, b] = (csize[r * CS + b * 128:
                                    r * CS + (b + 1) * 128] > 0)
    percore.update(xp1i=xp1i, xmskB=xmskB)

    # ---- edge2 schedule ----
    src2, dst2 = ei2[0], ei2[1]
    order2 = np.argsort(dst2, kind='stable')
    s_src2, s_dst2 = src2[order2], dst2[order2]
    deg2 = np.bincount(dst2, minlength=C1P).astype(np.float32)
    dinv2_full = (1.0 / np.maximum(deg2, 1.0)).astype(np.float32)
    dinv2_of_edge = dinv2_full[s_dst2]

    def e2row(c):  # row of cluster c in e2o row view ((CS+1) rows per rank)
        return (c // CS) * (CS + 1) + (c % CS)

    eblk2 = [[None] * NBP for _ in range(R)]
    for r in range(R):
        for b in range(NBP):
            lo, hi = r * CS + b * 128, r * CS + (b + 1) * 128
            eblk2[r][b] = (np.searchsorted(s_dst2, lo),
                           np.searchsorted(s_dst2, hi))
    S2 = [max(1, max(_ceil(eblk2[r][b][1] - eblk2[r][b][0], 128)
                     for r in range(R))) for b in range(NBP)]
    S2tot = sum(S2)
    src2g = np.zeros((R, 128, S2tot), np.int32)
    dst2loc = np.full((R, 128, S2tot), -1, np.int32)
    d2lrow = np.full((R, 1, 128 * S2tot), -1, np.int32)
    dinv2E = np.zeros((R, 128, S2tot), np.float32)
    t0 = 0
    for b in range(NBP):
        for r in range(R):
            i0, i1 = eblk2[r][b]
            ne = i1 - i0
            nn = 128 * S2[b]
            sg = np.full(nn, CS, np.int32)  # pad -> rank-0 zero row
            dl = np.full(nn, -1, np.int32)
            dvv = np.zeros(nn, np.float32)
            sg[:ne] = np.fromiter((e2row(c) for c in s_src2[i0:i1]),
                                  np.int32, ne)
            dl[:ne] = s_dst2[i0:i1] - (r * CS + b * 128)
            dvv[:ne] = dinv2_of_edge[i0:i1]
            src2g[r, :, t0:t0 + S2[b]] = sg.reshape(S2[b], 128).T
            dst2loc[r, :, t0:t0 + S2[b]] = dl.reshape(S2[b], 128).T
            d2lrow[r, 0, 128 * t0:128 * (t0 + S2[b])] = dl
            dinv2E[r, :, t0:t0 + S2[b]] = dvv.reshape(S2[b], 128).T
        t0 += S2[b]
    p["S2"] = S2
    percore.update(src2g=src2g, dst2loc=dst2loc, d2lrow=d2lrow, dinv2E=dinv2E)

    # ---- host-only int chains: batchp, batch2, counts ----
    NEG = np.int64(-10**9)
    bp = np.full(C1, NEG, np.int64)
    np.maximum.at(bp, cl1, batch.astype(np.int64))
    batchp = np.maximum(bp, 0).astype(np.int32)
    b2 = np.full(C2, NEG, np.int64)
    np.maximum.at(b2, cl2, batchp.astype(np.int64))
    batch2 = np.maximum(b2, 0).astype(np.int32)
    cntb = np.bincount(batch2, minlength=B).astype(np.float32)
    cntb_inv = (1.0 / np.maximum(cntb, 1.0)).astype(np.float32)

    # ---- pool2 schedule (cluster2 over C1 rows) ----
    c2order = np.argsort(cl2, kind='stable')
    c2_rows, c2_cl = c2order.astype(np.int32), cl2[c2order]
    c2size = np.bincount(cl2, minlength=C2P).astype(np.float32)

    def x2row(c1r):
        r = c1r // CS
        return r * (CS + 1) + (c1r - r * CS)

    # C2 cluster -> slot assignment is only referenced via x3i and selb, so
    # sort each rank's clusters by size (desc) before slotting: the per-block
    # layer count K2[b] = max cluster size in the block collapses for all but
    # the first block.
    K2 = []
    lay2 = [[] for _ in range(R)]
    selb = np.zeros((R, 128, B * NB2), np.float32)
    rank_slots = []
    for r in range(R):
        lo, hi = r * C2S, (r + 1) * C2S
        i0 = np.searchsorted(c2_cl, lo)
        i1 = np.searchsorted(c2_cl, hi)
        rows_by_c = {}
        for cr, cc in zip(c2_rows[i0:i1], c2_cl[i0:i1]):
            rows_by_c.setdefault(int(cc), []).append(int(cr))
        order = sorted(rows_by_c.keys(), key=lambda c: -len(rows_by_c[c]))
        slots = order + [-1] * (C2S - len(order))
        rank_slots.append((slots, rows_by_c))
    for b in range(NB2):
        kb = 1
        for r in range(R):
            slots, rows_by_c = rank_slots[r]
            for q in range(128):
                c = slots[b * 128 + q]
                if c >= 0:
                    kb = max(kb, len(rows_by_c[c]))
        K2.append(kb)
        for r in range(R):
            slots, rows_by_c = rank_slots[r]
            lt = np.full((kb, 128), sent2, np.int64)
            for q in range(128):
                c = slots[b * 128 + q]
                if c < 0:
                    continue
                for j, rr in enumerate(rows_by_c[c]):
                    lt[j, q] = x2row(rr)
                bv = int(batch2[c])
                selb[r, q, b * B + bv] = cntb_inv[bv]
            lay2[r].append(lt)
    x3i = np.stack([np.concatenate(lay2[r], 0).T.astype(np.int32)
                    for r in range(R)])
    p["K2"] = K2
    percore.update(x3i=x3i, selb=selb)

    return p, shared, percore


# ---------------------------------------------------------------------------
# device program
# ---------------------------------------------------------------------------

def elu(nc, pool, out, s, P, Fd):
    zneg = pool.tile([P, Fd], F32, tag="elu_zneg")
    nc.vector.tensor_scalar(zneg[:], s, 0.0, None, OP.min)
    ex = pool.tile([P, Fd], F32, tag="elu_ex")
    nc.scalar.activation(ex[:], zneg[:], ACT.Exp)
    zpos = pool.tile([P, Fd], F32, tag="elu_zpos")
    nc.vector.tensor_scalar(zpos[:], s, 0.0, -1.0, OP.max, OP.add)
    nc.vector.tensor_tensor(out, zpos[:], ex[:], OP.add)


def build_gnn(tc, outs, ins, p):
    nc = tc.nc
    R = p["R"]
    NB1, NBP, NB2 = p["NB1"], p["NBP"], p["NB2"]
    NS, CS = p["NS"], p["CS"]
    h1, co1, ci2, co2 = p["h1"], p["co1"], p["ci2"], p["co2"]
    FV, B, NCLS, FCH = p["FV"], p["B"], p["NCLS"], p["FCH"]
    W1 = FV * (h1 + 1)      # 156
    W2 = ci2 * (h1 + 1)     # 832
    L1, L2, L25, L3 = p["L1"], p["L2"], p["L25"], p["L3"]
    RW = p["RW"]
    S1tot, SPtot, S2tot = sum(p["S1"]), sum(p["SP"]), sum(p["S2"])
    K1tot, K2tot = sum(p["K1"]), sum(p["K2"])

    y = outs["y"]

    # internal DRAM
    e1i = nc.dram_tensor("e1i", [L1], F32, kind="Internal")
    e1o = nc.dram_tensor("e1o", [R * L1], F32, kind="Internal",
                         addr_space="Shared")
    e2i = nc.dram_tensor("e2i", [L2], F32, kind="Internal")
    e2o = nc.dram_tensor("e2o", [R * L2], F32, kind="Internal",
                         addr_space="Shared")
    egi = nc.dram_tensor("egi", [512], F32, kind="Internal")
    ego = nc.dram_tensor("ego", [R * 512], F32, kind="Internal",
                         addr_space="Shared")
    e25i = nc.dram_tensor("e25i", [L25], F32, kind="Internal")
    e25o = nc.dram_tensor("e25o", [R * L25], F32, kind="Internal",
                          addr_space="Shared")
    e3i = nc.dram_tensor("e3i", [L3], F32, kind="Internal")
    e3o = nc.dram_tensor("e3o", [R * L3], F32, kind="Internal",
                         addr_space="Shared")
    rdram = nc.dram_tensor("rdram", [1, 1], F32, kind="Internal")
    crdram = nc.dram_tensor("crdram", [1, 25], F32, kind="Internal")

    e1i_x1 = e1i.rearrange("(n c) -> n c", c=32)
    e1o_x1 = e1o.rearrange("(n c) -> n c", c=32)
    e2i_row = e2i.rearrange("(n c) -> n c", c=RW)
    e2o_row = e2o.rearrange("(n c) -> n c", c=RW)
    e25i_x2 = e25i.rearrange("(n c) -> n c", c=64)
    e25o_x2 = e25o.rearrange("(n c) -> n c", c=64)
    e3o_v = e3o.rearrange("(r f c) -> f r c", f=64, c=B)

    rg = [list(range(R))]

    with (
        tc.tile_pool(name="cp", bufs=1) as cp,
        tc.tile_pool(name="ri", bufs=1) as ri,
        tc.tile_pool(name="rd", bufs=1) as rd,
        tc.tile_pool(name="st", bufs=2) as st,
        tc.tile_pool(name="fp", bufs=2) as fp,
        tc.tile_pool(name="psU", bufs=1, space="PSUM") as psU,
        tc.tile_pool(name="psT", bufs=2, space="PSUM") as psT,
        tc.tile_pool(name="psZ", bufs=2, space="PSUM") as psZ,
    ):
        # ---- constants ----
        iot = cp.tile([128, 128], I32)
        nc.gpsimd.iota(iot[:], pattern=[[1, 128]], base=0, channel_multiplier=0)
        iotP = cp.tile([128, 1], I32)
        nc.gpsimd.iota(iotP[:], pattern=[[0, 1]], base=0, channel_multiplier=1)
        ident = cp.tile([128, 128], F32)
        make_identity(nc, ident[:])

        def load_const(name, shape):
            t = cp.tile(list(shape), F32, tag=name)
            nc.sync.dma_start(t[:], ins[name][:])
            return t

        w1a = load_const("w1a_aug", (4, h1))
        w1aB = cp.tile([128, 4 * h1], F32)
        for c in range(4):
            nc.sync.dma_start(
                w1aB[:, c * h1:(c + 1) * h1],
                ins["w1a_aug"][c:c + 1, :].to_broadcast([128, h1]))
        w2a = load_const("w2a_aug", (4, h1))
        wb1a = cp.tile([128, co1], F32)
        nc.sync.dma_start(wb1a[:], ins["wbig1"][0:128, :])
        wb1b = cp.tile([W1 - 128, co1], F32)
        nc.sync.dma_start(wb1b[:], ins["wbig1"][128:W1, :])
        wb2 = []
        for j in range(_ceil(W2, 128)):
            r0, r1 = j * 128, min((j + 1) * 128, W2)
            t = cp.tile([r1 - r0, co2], F32, tag=f"wb2_{j}", name=f"wb2_{j}")
            nc.sync.dma_start(t[:], ins["wbig2"][r0:r1, :])
            wb2.append(t)
        root1 = load_const("root1_aug", (FV + 1, co1))
        root2 = load_const("root2_aug", (ci2 + 1, co2))
        fc1w = load_const("fc1_w", (co2, FCH))
        fc1b = load_const("fc1_b", (FCH, 1))
        fc2w = load_const("fc2_w", (FCH, NCLS))
        fc2b = load_const("fc2_b", (NCLS, 1))

        # c_row = 0.5*sum(w2a rows) + b2a, computed on device
        halfones = load_const("halfones", (4, 1))
        crps = psT.tile([1, h1], F32, tag="pscr")
        nc.tensor.matmul(crps[:], lhsT=halfones[:], rhs=w2a[:],
                         start=True, stop=True)
        c_row1 = cp.tile([1, h1], F32)
        nc.vector.tensor_copy(c_row1[:], crps[:])
        nc.sync.dma_start(crdram[:], c_row1[:])
        c_row = cp.tile([128, h1], F32)
        nc.sync.dma_start(c_row[:], crdram[0:1, :].to_broadcast([128, h1]))

        # ---- resident schedule tensors ----
        def load_idx(name, shape, dt=I32):
            t = ri.tile(list(shape), dt, tag=name, name=name)
            nc.sync.dma_start(t[:], ins[name][:])
            return t

        sib = load_idx("src1i", (128, S1tot))
        dlb = load_idx("dst1loc", (128, S1tot))
        dvE = load_idx("dinv1E", (128, S1tot), F32)
        ccb = load_idx("clloc", (128, SPtot))
        wcb = load_idx("wcnt", (128, SPtot), F32)
        xpi = load_idx("xp1i", (128, K1tot))
        s2g = load_idx("src2g", (128, S2tot))
        d2l = load_idx("dst2loc", (128, S2tot))
        dv2E = load_idx("dinv2E", (128, S2tot), F32)
        x3b = load_idx("x3i", (128, K2tot))
        slb = load_idx("selb", (128, B * NB2), F32)
        xmb = load_idx("xmskB", (128, NBP), F32)
        eabR = load_idx("eabE", (128, 3 * S1tot), F32)
        posR = load_idx("pos_cl", (128, 4 * SPtot), F32)

        # ---- resident data ----
        tp_all = rd.tile([128, 29 * NBP], F32)
        gsrc = rd.tile([128, RW * S2tot], F32)
        qall = rd.tile([128, 29 * S2tot], F32)
        gacc = rd.tile([128, 1], F32)
        nc.vector.memset(gacc[:], 0.0)

        # ================= P1: conv1 =================
        t0 = 0
        for b in range(NB1):
            Sb = p["S1"][b]
            ea3 = eabR[:, 3 * t0:3 * (t0 + Sb)].rearrange(
                "p (s c) -> p s c", c=3)
            xga = st.tile([128, 8 * Sb], F32, tag="xga", bufs=4)
            xg3 = xga[:].rearrange("p (s c) -> p s c", c=8)
            for s in range(Sb):
                nc.gpsimd.indirect_dma_start(
                    out=xg3[:, s, :], out_offset=None, in_=ins["x_em"][:],
                    in_offset=bass.IndirectOffsetOnAxis(
                        ap=sib[:, t0 + s:t0 + s + 1], axis=0))
            nc.vector.tensor_tensor(
                xg3, xg3,
                dvE[:, t0:t0 + Sb, None].to_broadcast([128, Sb, 8]), OP.mult)
            hh = st.tile([128, h1 * Sb], F32, tag="hh", bufs=2)
            hh3 = hh[:].rearrange("p (s k) -> p s k", k=h1)
            htmp = st.tile([128, h1 * Sb], F32, tag="htmp", bufs=2)
            ht3 = htmp[:].rearrange("p (s k) -> p s k", k=h1)
            nc.vector.tensor_tensor(
                hh3, ea3[:, :, 0:1].to_broadcast([128, Sb, h1]),
                w1aB[:, None, 0:h1].to_broadcast([128, Sb, h1]), OP.mult)
            nc.vector.tensor_tensor(
                ht3, ea3[:, :, 1:2].to_broadcast([128, Sb, h1]),
                w1aB[:, None, h1:2 * h1].to_broadcast([128, Sb, h1]), OP.mult)
            nc.vector.tensor_tensor(hh3, hh3, ht3, OP.add)
            nc.vector.tensor_tensor(
                ht3, ea3[:, :, 2:3].to_broadcast([128, Sb, h1]),
                w1aB[:, None, 2 * h1:3 * h1].to_broadcast([128, Sb, h1]),
                OP.mult)
            nc.vector.tensor_tensor(hh3, hh3, ht3, OP.add)
            nc.vector.tensor_tensor(
                hh3, hh3,
                w1aB[:, None, 3 * h1:4 * h1].to_broadcast([128, Sb, h1]),
                OP.add)
            haug = st.tile([128, 26 * Sb], F32, tag="haug")
            ha3 = haug[:].rearrange("p (s k) -> p s k", k=26)
            nc.vector.memset(ha3[:, :, h1:h1 + 1], 1.0)
            nc.scalar.activation(ha3[:, :, 0:h1], hh3, ACT.Relu)
            sel = st.tile([128, 128 * Sb], F32, tag="sel", bufs=1)
            sel3 = sel[:].rearrange("p (s q) -> p s q", q=128)
            nc.vector.tensor_tensor(
                sel3, iot[:, None, :].to_broadcast([128, Sb, 128]),
                dlb[:, t0:t0 + Sb, None].to_broadcast([128, Sb, 128]),
                OP.is_equal)
            u = st.tile([128, W1 * Sb], F32, tag="u", bufs=1)
            u4 = u[:].rearrange("p (s i k) -> p s i k", i=FV, k=26)
            nc.vector.tensor_tensor(
                u4, ha3[:, :, None, :].to_broadcast([128, Sb, FV, 26]),
                xg3[:, :, 0:FV, None].to_broadcast([128, Sb, FV, 26]), OP.mult)
            U = psU.tile([128, W1], F32, tag="U1")
            for s in range(Sb):
                nc.tensor.matmul(U[:], lhsT=sel[:, 128 * s:128 * (s + 1)],
                                 rhs=u[:, W1 * s:W1 * (s + 1)],
                                 start=(s == 0), stop=(s == Sb - 1))
            Usb = fp.tile([128, W1], F32, tag="Usb", bufs=1)
            nc.vector.tensor_copy(Usb[:], U[:])
            tr1 = psT.tile([128, 128], F32, tag="pscr")
            nc.tensor.transpose(tr1[:], Usb[:, 0:128], ident[:])
            Ut1 = fp.tile([128, 128], F32, tag="Ut1", bufs=1)
            nc.vector.tensor_copy(Ut1[:], tr1[:])
            tr2 = psT.tile([W1 - 128, 128], F32, tag="pscr")
            nc.tensor.transpose(tr2[:], Usb[:, 128:W1], ident[:])
            Ut2 = fp.tile([W1 - 128, 128], F32, tag="Ut2", bufs=1)
            nc.vector.tensor_copy(Ut2[:], tr2[:])
            xat = fp.tile([FV + 1, 128], F32, tag="xat")
            nc.sync.dma_start(xat[:], ins["xaugT"][:, b * 128:(b + 1) * 128])
            z = psZ.tile([co1, 128], F32, tag="cagg")
            nc.tensor.matmul(z[:], lhsT=wb1a[:], rhs=Ut1[:], start=True,
                             stop=False)
            nc.tensor.matmul(z[:], lhsT=wb1b[:], rhs=Ut2[:], start=False,
                             stop=False)
            nc.tensor.matmul(z[:], lhsT=root1[:], rhs=xat[:], start=False,
                             stop=True)
            x1f = fp.tile([co1, 128], F32, tag="x1f")
            elu(nc, fp, x1f[:], z[:], co1, 128)
            x1p = psT.tile([128, co1], F32, tag="pscr")
            nc.tensor.transpose(x1p[:], x1f[:], ident[0:co1, 0:co1])
            x1e = fp.tile([128, co1], F32, tag="x1e")
            nc.vector.tensor_copy(x1e[:], x1p[:])
            nc.sync.dma_start(e1i_x1[b * 128:(b + 1) * 128, :], x1e[:])
            t0 += Sb

        padr1 = fp.tile([1, 32], F32, tag="padr1")
        nc.vector.memset(padr1[:], -1.0)
        nc.sync.dma_start(e1i[32 * NS:32 * NS + 32], padr1[:])

        # ================= E1: AllGather x1 =================
        nc.gpsimd.collective_compute(
            "AllGather", OP.bypass, replica_groups=rg, ins=[e1i[:]],
            outs=[e1o[:]])

        # ================= P2: posp + t rows =================
        t0 = 0
        for b in range(NBP):
            Sb = p["SP"][b]
            wsel = st.tile([128, 128 * Sb], F32, tag="wsel")
            ws3 = wsel[:].rearrange("p (s q) -> p s q", q=128)
            nc.vector.tensor_tensor(
                ws3, iot[:, None, :].to_broadcast([128, Sb, 128]),
                ccb[:, t0:t0 + Sb, None].to_broadcast([128, Sb, 128]),
                OP.is_equal)
            nc.vector.tensor_tensor(
                ws3, ws3,
                wcb[:, t0:t0 + Sb, None].to_broadcast([128, Sb, 128]), OP.mult)
            PP = psT.tile([128, 4], F32, tag="ppacc", bufs=1)
            pg3 = posR[:, 4 * t0:4 * (t0 + Sb)].rearrange(
                "p (s c) -> p s c", c=4)
            for s in range(Sb):
                nc.tensor.matmul(PP[:], lhsT=wsel[:, 128 * s:128 * (s + 1)],
                                 rhs=pg3[:, s, :], start=(s == 0),
                                 stop=(s == Sb - 1))
            ppsb = st.tile([128, 4], F32, tag="ppsb")
            nc.vector.tensor_copy(ppsb[:], PP[:])
            ppt_ps = psT.tile([4, 128], F32, tag="pscr")
            nc.tensor.transpose(ppt_ps[:], ppsb[:], ident[:])
            ppt = st.tile([4, 128], F32, tag="ppt")
            nc.vector.tensor_copy(ppt[:], ppt_ps[:])
            tps = psT.tile([128, h1], F32, tag="pscr")
            nc.tensor.matmul(tps[:], lhsT=ppt[0:3, :], rhs=w2a[0:3, :],
                             start=True, stop=True)
            nc.vector.tensor_copy(tp_all[:, 29 * b:29 * b + 25], tps[:])
            nc.vector.tensor_copy(tp_all[:, 29 * b + 25:29 * b + 29], ppsb[:])
            nc.sync.dma_start(e2i_row[b * 128:(b + 1) * 128, 32:61],
                              tp_all[:, 29 * b:29 * (b + 1)])
            t0 += Sb

        # ================= P4: pool1 xp =================
        xpa_all = rd.tile([128, 32 * NBP], F32)
        t0 = 0
        for b in range(NBP):
            Kb = p["K1"][b]
            xg1 = st.tile([128, 32 * Kb], F32, tag="xg1", bufs=3)
            xg13 = xg1[:].rearrange("p (k c) -> p k c", c=32)
            for j in range(Kb):
                nc.gpsimd.indirect_dma_start(
                    out=xg13[:, j, :], out_offset=None, in_=e1o_x1[:],
                    in_offset=bass.IndirectOffsetOnAxis(
                        ap=xpi[:, t0 + j:t0 + j + 1], axis=0))
            xpm = st.tile([128, 32], F32, tag="xpm")
            nc.vector.tensor_reduce(
                xpm[:], xg1[:].rearrange("p (k c) -> p c k", c=32), AX, OP.max)
            nc.vector.tensor_tensor(
                xpm[:], xpm[:], xmb[:, b:b + 1].to_broadcast([128, 32]),
                OP.mult)
            nc.vector.tensor_copy(xpa_all[:, 32 * b:32 * (b + 1)], xpm[:])
            nc.sync.dma_start(e2i_row[b * 128:(b + 1) * 128, 0:32], xpm[:])
            t0 += Kb

        padr2e = fp.tile([1, RW], F32, tag="padr2e")
        nc.vector.memset(padr2e[:], 0.0)
        nc.sync.dma_start(e2i[RW * CS:RW * (CS + 1)], padr2e[:])

        # ================= E2: AllGather [xp|t|posp] rows =================
        nc.gpsimd.collective_compute(
            "AllGather", OP.bypass, replica_groups=rg, ins=[e2i[:]],
            outs=[e2o[:]])

        # ================= passA: gather rows, q = rows_s - rows_d, gmax ====
        t0 = 0
        for b in range(NBP):
            Sb = p["S2"][b]
            g3 = gsrc[:].rearrange("p (s c) -> p s c", c=RW)
            for s in range(Sb):
                nc.gpsimd.indirect_dma_start(
                    out=g3[:, t0 + s, :], out_offset=None, in_=e2o_row[:],
                    in_offset=bass.IndirectOffsetOnAxis(
                        ap=s2g[:, t0 + s:t0 + s + 1], axis=0))
            d2lb = st.tile([128, 128 * Sb], I32, tag="d2lb", bufs=1)
            nc.sync.dma_start(
                d2lb[:], ins["d2lrow"][0:1, 128 * t0:128 * (t0 + Sb)]
                .to_broadcast([128, 128 * Sb]))
            selT = st.tile([128, 128 * Sb], F32, tag="selT", bufs=1)
            sT3 = selT[:].rearrange("p (s q) -> p s q", q=128)
            nc.vector.tensor_tensor(
                sT3, iotP[:, :, None].to_broadcast([128, Sb, 128]),
                d2lb[:].rearrange("p (s q) -> p s q", q=128), OP.is_equal)
            q3 = qall[:].rearrange("p (s c) -> p s c", c=29)
            for s in range(Sb):
                td = psT.tile([128, 29], F32, tag="pscr")
                nc.tensor.matmul(td[:], lhsT=selT[:, 128 * s:128 * (s + 1)],
                                 rhs=tp_all[:, 29 * b:29 * (b + 1)],
                                 start=True, stop=True)
                nc.vector.tensor_tensor(
                    q3[:, t0 + s, :], g3[:, t0 + s, 32:61], td[:], OP.subtract)
            rb = st.tile([128, 1], F32, tag="rb")
            nc.vector.tensor_reduce(
                rb[:], q3[:, t0:t0 + Sb, 25:29], AXY, OP.max,
                apply_absolute_value=True)
            nc.vector.tensor_tensor(gacc[:], gacc[:], rb[:], OP.max)
            t0 += Sb

        # local gmax -> AllGather -> rec = 1/(2*gmax)
        gtp = psT.tile([1, 128], F32, tag="pscr")
        nc.tensor.transpose(gtp[:], gacc[:], ident[:])
        gts = fp.tile([1, 128], F32, tag="gts")
        nc.vector.tensor_copy(gts[:], gtp[:])
        gmx = fp.tile([1, 1], F32, tag="gmx")
        nc.vector.tensor_reduce(gmx[:], gts[:], AX, OP.max)
        gmxrow = fp.tile([1, 512], F32, tag="gmxrow")
        nc.vector.tensor_copy(gmxrow[:], gmx[:].to_broadcast([1, 512]))
        nc.sync.dma_start(egi[:], gmxrow[:])
        nc.gpsimd.collective_compute(
            "AllGather", OP.bypass, replica_groups=rg, ins=[egi[:]],
            outs=[ego[:]])
        g8 = fp.tile([1, R * 512], F32, tag="g8")
        nc.sync.dma_start(g8[:], ego[:])
        gm1 = fp.tile([1, 1], F32, tag="gm1")
        nc.vector.tensor_reduce(gm1[:], g8[:], AX, OP.max)
        rec = fp.tile([1, 1], F32, tag="rec")
        nc.vector.reciprocal(rec[:], gm1[:])
        nc.vector.tensor_scalar(rec[:], rec[:], 0.5, None, OP.mult)
        nc.sync.dma_start(rdram[:], rec[:])
        rcol = rd.tile([128, 1], F32)
        nc.sync.dma_start(rcol[:], rdram[0:1, 0:1].to_broadcast([128, 1]))

        # ================= passB: conv2 =================
        t0 = 0
        for b in range(NBP):
            Sb = p["S2"][b]
            q3 = qall[:].rearrange("p (s c) -> p s c", c=29)
            g3 = gsrc[:].rearrange("p (s c) -> p s c", c=RW)
            sel2 = st.tile([128, 128 * Sb], F32, tag="sel2", bufs=1)
            s23 = sel2[:].rearrange("p (s q) -> p s q", q=128)
            nc.vector.tensor_tensor(
                s23, iot[:, None, :].to_broadcast([128, Sb, 128]),
                d2l[:, t0:t0 + Sb, None].to_broadcast([128, Sb, 128]),
                OP.is_equal)
            xps = st.tile([128, 32 * Sb], F32, tag="xps")
            xp3 = xps[:].rearrange("p (s c) -> p s c", c=32)
            nc.vector.tensor_tensor(
                xp3, g3[:, t0:t0 + Sb, 0:32],
                dv2E[:, t0:t0 + Sb, None].to_broadcast([128, Sb, 32]), OP.mult)
            hh2 = st.tile([128, h1 * Sb], F32, tag="hh2")
            h23 = hh2[:].rearrange("p (s k) -> p s k", k=h1)
            nc.vector.tensor_tensor(
                h23, q3[:, t0:t0 + Sb, 0:h1],
                rcol[:, :, None].to_broadcast([128, Sb, h1]), OP.mult)
            nc.vector.tensor_tensor(
                h23, h23,
                c_row[:, None, :].to_broadcast([128, Sb, h1]), OP.add)
            haug2 = st.tile([128, 26 * Sb], F32, tag="haug2")
            ha23 = haug2[:].rearrange("p (s k) -> p s k", k=26)
            nc.vector.memset(ha23[:, :, h1:h1 + 1], 1.0)
            nc.scalar.activation(ha23[:, :, 0:h1], h23, ACT.Relu)
            u2 = st.tile([128, W2 * Sb], F32, tag="u2", bufs=1)
            u24 = u2[:].rearrange("p (s i k) -> p s i k", i=ci2, k=26)
            nc.vector.tensor_tensor(
                u24, ha23[:, :, None, :].to_broadcast([128, Sb, ci2, 26]),
                xp3[:, :, :, None].to_broadcast([128, Sb, ci2, 26]), OP.mult)
            U1 = psU.tile([128, 512], F32, tag="U2a")
            U2 = psU.tile([128, W2 - 512], F32, tag="U2b")
            for s in range(Sb):
                st_, sp_ = (s == 0), (s == Sb - 1)
                nc.tensor.matmul(U1[:], lhsT=sel2[:, 128 * s:128 * (s + 1)],
                                 rhs=u2[:, W2 * s:W2 * s + 512],
                                 start=st_, stop=sp_)
                nc.tensor.matmul(U2[:], lhsT=sel2[:, 128 * s:128 * (s + 1)],
                                 rhs=u2[:, W2 * s + 512:W2 * (s + 1)],
                                 start=st_, stop=sp_)
            U1sb = fp.tile([128, 512], F32, tag="U1sb", bufs=1)
            nc.vector.tensor_copy(U1sb[:], U1[:])
            U2sb = fp.tile([128, W2 - 512], F32, tag="U2sb", bufs=1)
            nc.vector.tensor_copy(U2sb[:], U2[:])
            z2 = psZ.tile([co2, 128], F32, tag="cagg")
            for j in range(7):
                c0 = 128 * j
                cw = min(128, W2 - c0)
                src_sb = U1sb if c0 < 512 else U2sb
                off = c0 if c0 < 512 else c0 - 512
                trj = psT.tile([128, 128], F32, tag="pscr")
                nc.tensor.transpose(trj[0:cw, :], src_sb[:, off:off + cw],
                                    ident[:])
                Utj = fp.tile([128, 128], F32, tag="Utj", bufs=1)
                nc.vector.tensor_copy(Utj[0:cw, :], trj[0:cw, :])
                nc.tensor.matmul(z2[:], lhsT=wb2[j][:], rhs=Utj[0:cw, :],
                                 start=(j == 0), stop=False)
            xtp = psT.tile([ci2, 128], F32, tag="pscr")
            nc.tensor.transpose(xtp[:], xpa_all[:, 32 * b:32 * (b + 1)],
                                ident[:])
            xpt = fp.tile([ci2 + 1, 128], F32, tag="xptb", bufs=1)
            nc.vector.tensor_copy(xpt[0:ci2, :], xtp[:])
            nc.vector.memset(xpt[ci2:ci2 + 1, :], 1.0)
            nc.tensor.matmul(z2[:], lhsT=root2[:], rhs=xpt[:],
                             start=False, stop=True)
            x2f = fp.tile([co2, 128], F32, tag="x2f")
            elu(nc, fp, x2f[:], z2[:], co2, 128)
            x2p = psT.tile([128, co2], F32, tag="pscr")
            nc.tensor.transpose(x2p[:], x2f[:], ident[0:co2, 0:co2])
            x2e = fp.tile([128, co2], F32, tag="x2e")
            nc.vector.tensor_copy(x2e[:], x2p[:])
            nc.sync.dma_start(e25i_x2[b * 128:(b + 1) * 128, :], x2e[:])
            t0 += Sb

        padr2 = fp.tile([1, 64], F32, tag="padr2")
        nc.vector.memset(padr2[:], -1.0)
        nc.sync.dma_start(e25i[64 * CS:64 * CS + 64], padr2[:])

        # ================= E25 =================
        nc.gpsimd.collective_compute(
            "AllGather", OP.bypass, replica_groups=rg, ins=[e25i[:]],
            outs=[e25o[:]])

        # ================= P6: pool2 + partial g =================
        gps = psZ.tile([co2, B], F32, tag="cagg")
        t0 = 0
        for b in range(NB2):
            Kb = p["K2"][b]
            xg2 = st.tile([128, 64 * Kb], F32, tag="xg2", bufs=3)
            xg23 = xg2[:].rearrange("p (k c) -> p k c", c=64)
            for j in range(Kb):
                nc.gpsimd.indirect_dma_start(
                    out=xg23[:, j, :], out_offset=None, in_=e25o_x2[:],
                    in_offset=bass.IndirectOffsetOnAxis(
                        ap=x3b[:, t0 + j:t0 + j + 1], axis=0))
            acc = st.tile([128, 64], F32, tag="acc2")
            nc.vector.tensor_reduce(
                acc[:], xg2[:].rearrange("p (k c) -> p c k", c=64), AX, OP.max)
            nc.tensor.matmul(gps[:], lhsT=acc[:], rhs=slb[:, b * B:(b + 1) * B],
                             start=(b == 0), stop=(b == NB2 - 1))
            t0 += Kb
        gsb = fp.tile([co2, B], F32, tag="gsb")
        nc.vector.tensor_copy(gsb[:], gps[:])
        nc.sync.dma_start(e3i.rearrange("(f c) -> f c", c=B)[:], gsb[:])

        # ================= E3 =================
        nc.gpsimd.collective_compute(
            "AllGather", OP.bypass, replica_groups=rg, ins=[e3i[:]],
            outs=[e3o[:]])

        # ================= P7: tail (replicated) =================
        t8 = fp.tile([co2, R * B], F32, tag="t8")
        nc.sync.dma_start(t8[:].rearrange("p (r c) -> p r c", c=B), e3o_v[:])
        gsum = fp.tile([co2, B], F32, tag="gsum")
        nc.vector.tensor_reduce(
            gsum[:], t8[:].rearrange("p (r c) -> p c r", c=B), AX, OP.add)
        z1p = psZ.tile([FCH, B], F32, tag="cagg")
        nc.tensor.matmul(z1p[:], lhsT=fc1w[:], rhs=gsum[:], start=True,
                         stop=True)
        z1 = fp.tile([FCH, B], F32, tag="z1")
        nc.scalar.activation(z1[:], z1p[:], ACT.Identity, bias=fc1b[:])
        h1t = fp.tile([FCH, B], F32, tag="h1t")
        elu(nc, fp, h1t[:], z1[:], FCH, B)
        z2p = psZ.tile([NCLS, B], F32, tag="cagg")
        nc.tensor.matmul(z2p[:], lhsT=fc2w[:], rhs=h1t[:], start=True,
                         stop=True)
        z2t = fp.tile([NCLS, B], F32, tag="z2t")
        nc.scalar.activation(z2t[:], z2p[:], ACT.Identity, bias=fc2b[:])
        ztp = psT.tile([B, NCLS], F32, tag="pscr")
        nc.tensor.transpose(ztp[:], z2t[:], ident[0:NCLS, 0:NCLS])
        zt = fp.tile([B, NCLS], F32, tag="zt")
        nc.vector.tensor_copy(zt[:], ztp[:])
        m = fp.tile([B, 1], F32, tag="m")
        nc.vector.tensor_reduce(m[:], zt[:], AX, OP.max)
        zs = fp.tile([B, NCLS], F32, tag="zs")
        nc.vector.tensor_tensor(zs[:], zt[:], m[:].to_broadcast([B, NCLS]),
                                OP.subtract)
        ex = fp.tile([B, NCLS], F32, tag="exf")
        ssum = fp.tile([B, 1], F32, tag="ssum")
        nc.scalar.activation(ex[:], zs[:], ACT.Exp, accum_out=ssum[:])
        lg = fp.tile([B, 1], F32, tag="lg")
        nc.scalar.activation(lg[:], ssum[:], ACT.Ln)
        out_t = fp.tile([B, NCLS], F32, tag="out_t")
        nc.vector.tensor_tensor(out_t[:], zs[:], lg[:].to_broadcast([B, NCLS]),
                                OP.subtract)
        nc.sync.dma_start(y[:], out_t[:])


# ---------------------------------------------------------------------------
# SPMD runner (PJRT via axon)
# ---------------------------------------------------------------------------

class SpmdRunner:
    def __init__(self, nc, n_cores):
        import jax
        from jax.sharding import Mesh, PartitionSpec
        from jax.experimental.shard_map import shard_map
        from concourse import bass2jax
        from concourse.bass2jax import _bass_exec_p, partition_id_tensor
        bass2jax.install_neuronx_cc_hook()
        self.jax = jax
        self.nc = nc
        self.n_cores = n_cores
        in_names, out_names, out_avals, zero_outs = [], [], [], []
        partition_name = (nc.partition_id_tensor.name
                          if nc.partition_id_tensor else None)
        for alloc in nc.m.functions[0].allocations:
            if not isinstance(alloc, mybir.MemoryLocationSet):
                continue
            name = alloc.memorylocations[0].name
            if alloc.kind == "ExternalInput":
                if name != partition_name:
                    in_names.append(name)
            elif alloc.kind == "ExternalOutput":
                out_names.append(name)
                shape = tuple(alloc.tensor_shape)
                dtype = mybir.dt.np(alloc.dtype)
                out_avals.append(jax.core.ShapedArray(shape, dtype))
                zero_outs.append(np.zeros(shape, dtype))
        self.in_names, self.out_names = in_names, out_names
        self.out_avals, self.zero_outs = out_avals, zero_outs
        n_params = len(in_names)
        n_outs = len(out_avals)
        all_in_names = list(in_names) + list(out_names)
        if partition_name is not None:
            all_in_names.append(partition_name)

        def _body(*args):
            operands = list(args)
            if partition_name is not None:
                operands.append(partition_id_tensor())
            outs = _bass_exec_p.bind(
                *operands, out_avals=tuple(out_avals),
                in_names=tuple(all_in_names),
                out_names=tuple(out_names), lowering_input_output_aliases=(),
                sim_require_finite=False, sim_require_nnan=False, nc=nc)
            return tuple(outs)

        devices = jax.devices()[:n_cores]
        mesh = Mesh(np.asarray(devices), ("core",))
        in_specs = (PartitionSpec("core"),) * (n_params + n_outs)
        out_specs = (PartitionSpec("core"),) * n_outs
        self.fn = jax.jit(
            shard_map(_body, mesh=mesh, in_specs=in_specs,
                      out_specs=out_specs, check_rep=False),
            keep_unused=True)
        self.n_params = n_params

    def prepare(self, in_maps):
        per_core = [[np.asarray(m[name]) for name in self.in_names]
                    for m in in_maps]
        concat_in = [
            np.concatenate([per_core[c][i] for c in range(self.n_cores)],
                           axis=0)
            for i in range(self.n_params)]
        concat_zeros = [
            np.zeros((self.n_cores * z.shape[0], *z.shape[1:]), z.dtype)
            for z in self.zero_outs]
        self.args = self.jax.device_put(concat_in + concat_zeros)

    def run(self):
        outs = self.fn(*self.args)
        self.jax.block_until_ready(outs)
        return outs

    def results(self, outs):
        return [
            {name: np.asarray(outs[i]).reshape(
                self.n_cores, *self.out_avals[i].shape)[c]
             for i, name in enumerate(self.out_names)}
            for c in range(self.n_cores)]


# ---------------------------------------------------------------------------
# kernel entry point
# ---------------------------------------------------------------------------

def _in_maps_from_prep(p, shared, percore):
    R = p["R"]
    maps = []
    for r in range(R):
        m = dict(shared)
        for k, v in percore.items():
            m[k] = v[r]
        maps.append(m)
    return maps


def build_nc(p, in_specs):
    install_tilefix()
    nc = bass.Bass(num_devices=p["R"])
    ins = {}
    for name, (shape, dt_) in in_specs.items():
        mdt = F32 if np.dtype(dt_) == np.float32 else I32
        ins[name] = nc.dram_tensor(name, list(shape), mdt,
                                   kind="ExternalInput")
    y = nc.dram_tensor("y", [p["B"], p["NCLS"]], F32, kind="ExternalOutput")
    with tile.TileContext(nc) as tc:
        build_gnn(tc, {"y": y}, ins, p)
    split_excess_waits(nc, limit=1)
    return nc


_CACHE = {}


def kernel(**inputs):
    p, shared, percore = prep(inputs, R=8)
    in_maps = _in_maps_from_prep(p, shared, percore)
    in_specs = {k: (v.shape, v.dtype) for k, v in in_maps[0].items()}
    key = tuple(sorted((k, tuple(s), str(d)) for k, (s, d) in in_specs.items()))
    if key not in _CACHE:
        nc = build_nc(p, in_specs)
        _CACHE[key] = SpmdRunner(nc, p["R"])
    runner = _CACHE[key]
    runner.prepare(in_maps)
    outs = runner.run()
    res = runner.results(outs)
    return res[0]["y"].astype(np.float32)


if __name__ == "__main__":
    pass
